# revision 1
# baseline (speedup 1.0000x reference)
"""CapsuleNetwork Trainium2 kernel (Bass/Tile), 8-core data parallel.

Math reformulation (validated vs reference in fp32, rel err ~3e-7):
  primary p = x @ Wp + bp, viewed [B, n=8, d=16]
  squash scales gp[b,n] = gamma(||p_n||^2),  gamma(q) = q/(1+q)/sqrt(q+1e-8)
  u_hat_n = gp_n * (p_n @ W_n)
  Routing only needs the per-sample Gram matrix
      G[b,n,m] = u_hat_n . u_hat_m = gp_n gp_m * (p_n K_nm p_m),
  with K = Wflat @ Wflat^T precomputed on host ([128,128]).
  Routing runs on [B,8]/[B,8,8] tensors; final
      v = sum_m w_m u_hat_m = (p .* w_bcast) @ Wflat,  w = gam_last*c_last*gp.

Per-core: 2048 rows = 16 tiles of 128, processed as 8 pairs (256-wide
matmuls).  Tiles are split into 2 groups; each group's routing (DVE-only,
on [128, wide] batched tensors) and stage-2 output overlap the other
group's PE-heavy stage 1.
"""

import numpy as np

import concourse.bass as bass
import concourse.tile as tile
from concourse import mybir
from concourse.bass_utils import run_bass_kernel_spmd
from concourse.vector_clock import ScopedClock

F32 = mybir.dt.float32
F32R = mybir.dt.float32r   # tf32-class PE format: 1 cyc/row when N>=256 (vs 4 for fp32)
AF = mybir.ActivationFunctionType
AX = mybir.AxisListType

N_CORES = 8
B_FULL, IN_DIM, OUT_DIM = 16384, 1024, 512
N_CAPS, CAP_DIM = 8, 16
ND = N_CAPS * CAP_DIM          # 128
B_CORE = B_FULL // N_CORES     # 2048
P = 128                        # partitions / tile rows
PB = 2 * P                     # batch per pair
K_CHUNKS = IN_DIM // P         # 8


def _patched_drain_and_barrier(self, tick_clock, wait_clock):
    # Walrus in this env allows at most ONE sem wait per instruction; the
    # stock tail drain accumulates one wait per live sem.  Collect waits on
    # a NOP, then re-emit one wait per NOP.
    nc = self.nc
    collector = nc.sync.nop()
    wait_clock.add_sem_waits(
        collector.ins, ScopedClock({None: tick_clock.global_clock})
    )
    si = collector.ins.sync_info
    waits = list(si.on_wait) if si is not None else []
    if len(waits) > 1:
        collector.ins.sync_info = mybir.SyncInfo(on_wait=waits[:1], on_update=[])
        for w in waits[1:]:
            n = nc.sync.nop()
            n.ins.sync_info = mybir.SyncInfo(on_wait=[w], on_update=[])
    nc.sync.drain()
    nc.all_engine_barrier()
    popped = nc._tile_sem_poison_stack.pop()
    assert popped is self._sem_poison
    nc.clear_and_free_semaphores(list(self.sems.allocated().values()))
    nc.all_engine_barrier()


tile.TileContext._drain_and_barrier = _patched_drain_and_barrier


def _split_multi_waits(nc):
    """Walrus here accepts at most one sem wait per instruction.  Tile's
    wait-assignment can attach several; split the extras onto single-wait
    NOPs inserted just before the instruction on the same engine."""
    k = 0
    for fn in nc.m.functions:
        for blk in fn.blocks:
            out = []
            for inst in blk.instructions:
                si = inst.sync_info
                if si is not None and len(si.on_wait) > 1:
                    waits = list(si.on_wait)
                    for w in waits[:-1]:
                        nop = mybir.InstNoOp(name=f"wsplit-{k}", ins=[], outs=[])
                        k += 1
                        nop.engine = inst.engine
                        nop.sync_info = mybir.SyncInfo(on_wait=[w], on_update=[])
                        nc.register_instruction(nop, overwrite=True)
                        out.append(nop)
                    inst.sync_info = mybir.SyncInfo(
                        on_wait=[waits[-1]], on_update=list(si.on_update)
                    )
                out.append(inst)
            blk.instructions = out

def build_nc(n_routing: int, n_tiles: int = B_CORE // P):
    assert n_tiles % 2 == 0
    nc = bass.Bass()
    rows = n_tiles * P
    n_pairs = n_tiles // 2
    if n_pairs >= 2:
        h = (n_pairs + 1) // 2
        groups = [list(range(0, h)), list(range(h, n_pairs))]
    else:
        groups = [list(range(n_pairs))]
    groups = [g for g in groups if g]

    # xt: host-pre-transposed x, block layout [blk, p, k, pp, b] so a
    # 2-pair block is one contiguous [128, 8*512] DMA
    n_blocks = (n_pairs + 1) // 2
    xt_ext = nc.declare_dram_parameter(
        "xt", [n_blocks * P, K_CHUNKS * 2 * PB], F32R, isOutput=False
    )
    # packed consts, one tensor: cols [0:128]=ident(F32 bits), [128]=bp,
    # [129:137]=nmask, [137:265]=mmask(rows 0:8), [265:1289]=wpc,
    # [1289:2313]=kt, [2313:2825]=wflat
    CPACK = 2825
    cp_ext = nc.declare_dram_parameter("cpack", [P, CPACK], F32R, isOutput=False)
    v_ext = nc.declare_dram_parameter("v", [rows, OUT_DIM], F32, isOutput=True)

    T = n_tiles

    with tile.TileContext(nc) as tc:
        with (
            tc.tile_pool(name="consts", bufs=1) as cpool,
            tc.tile_pool(name="persist", bufs=1) as ppool,
            tc.tile_pool(name="xin", bufs=2) as xpool,
            tc.tile_pool(name="s1sb", bufs=2) as s1pool,
            tc.tile_pool(name="rt", bufs=1) as rt,
            tc.tile_pool(name="s2sb", bufs=2) as s2sb,
        ):
            cp = cpool.tile([P, CPACK], F32R)
            nc.sync.dma_start(cp[:, 0:1289], cp_ext[:, 0:1289])
            ident = cp[:, 0:128].bitcast(F32)
            bp_sb = cp[:, 128:129].bitcast(F32)
            nmask_r = cp[:, 129:137]
            mmask_r = cp[0:8, 137:265]
            wpc_r = cp[:, 265:1289]
            eps_sb = cpool.tile([P, 1], F32)
            nc.gpsimd.memset(eps_sb[:], 1e-8)

            # xt prefetch for the first two 2-pair blocks, between const DMAs
            x_pre = {}

            def load_xt_block(blk):
                xt_sb = s1pool.tile([P, K_CHUNKS, 2 * PB], F32R, tag="xt")
                xin = xt_ext[blk * P:(blk + 1) * P, :].rearrange(
                    "p (k c) -> p k c", k=K_CHUNKS
                )
                if blk == 0:
                    nc.scalar.dma_start(xt_sb[:, 0:2, :], xin[:, 0:2, :])
                    nc.scalar.dma_start(xt_sb[:, 2:5, :], xin[:, 2:5, :])
                    nc.scalar.dma_start(
                        xt_sb[:, 5:K_CHUNKS, :], xin[:, 5:K_CHUNKS, :]
                    )
                elif blk == 1:
                    nc.scalar.dma_start(xt_sb[:, 0:4, :], xin[:, 0:4, :])
                    nc.scalar.dma_start(
                        xt_sb[:, 4:K_CHUNKS, :], xin[:, 4:K_CHUNKS, :]
                    )
                else:
                    nc.scalar.dma_start(xt_sb[:], xin)
                return xt_sb

            for blk in range(min(2, n_blocks)):
                x_pre[blk] = load_xt_block(blk)

            nc.sync.dma_start(cp[:, 1289:CPACK], cp_ext[:, 1289:CPACK])
            kt_r = cp[:, 1289:2313]
            wflat_r = cp[:, 2313:2825]

            # Wide per-core accumulators; sq/g/w/gam are PER GROUP so group
            # chains share no tensors at all
            p_all = ppool.tile([P, T * ND], F32R)       # primary^T, [:, t*128:...] = [(nd), b]
            t_of_g = {}
            sq_of, g_of, w_of, gam_of = {}, {}, {}, {}
            for gi, g in enumerate(groups):
                tg0, tg1 = 2 * g[0], 2 * (g[-1] + 1)
                t_of_g[gi] = (tg0, tg1)
                Tg = tg1 - tg0
                sq_of[tg0] = ppool.tile([P, Tg * N_CAPS], F32, name=f"sqg{tg0}")
                g_of[tg0] = ppool.tile([P, Tg * 64], F32, name=f"gg{tg0}")
                w_of[tg0] = ppool.tile([P, Tg * N_CAPS], F32, name=f"wg{tg0}")
                gam_of[tg0] = ppool.tile([P, Tg], F32, name=f"gamg{tg0}")

            # ---------------- Stage 1 (per pair of tiles) ----------------
            def grp_base(pr):
                for gi, g in enumerate(groups):
                    if pr in g:
                        return t_of_g[gi][0]
                raise AssertionError

            xt_blocks = {}

            def stage1(pr, pools):
                xt_ps_pool, p_ps_pool, z_ps_pool, sg_ps_pool = pools
                t0 = 2 * pr
                tb = grp_base(pr)
                g_all = g_of[tb]
                sq_all = sq_of[tb]
                blk, half = pr // 2, pr % 2
                if half == 0:
                    if blk in x_pre:
                        xt_blocks[blk] = x_pre[blk]
                    else:
                        xt_blocks[blk] = load_xt_block(blk)
                xt_sb = xt_blocks[blk][:, :, half * PB:(half + 1) * PB]

                p_ps = p_ps_pool.tile([P, PB], F32, tag="pps")
                for k in range(K_CHUNKS):
                    nc.tensor.matmul(
                        p_ps[:],
                        wpc_r[:, k * P:(k + 1) * P],
                        xt_sb[:, k, :],
                        start=(k == 0),
                        stop=(k == K_CHUNKS - 1),
                    )
                p_sb = p_all[:, t0 * ND:(t0 + 2) * ND]   # [(nd), 256] fp32r
                nc.scalar.activation(
                    p_sb, p_ps[:], AF.Identity, bias=bp_sb[:, 0:1], scale=1.0
                )
                p_f = p_sb.bitcast(F32)

                p2_sb = s1pool.tile([P, PB], F32R, tag="p2")
                nc.gpsimd.tensor_mul(p2_sb[:], p_f, p_f)

                # sq & G share one PSUM tile: [:, 0:128] = G(2 tiles), [:, 128:144] = sq
                sg = sg_ps_pool.tile([P, 144], F32, tag="sg")
                for ti in range(2):
                    nc.tensor.matmul(
                        sg[:, 128 + ti * 8:128 + (ti + 1) * 8],
                        p2_sb[:, ti * P:(ti + 1) * P],
                        nmask_r[:],
                        start=True,
                        stop=True,
                    )

                e_sb = s1pool.tile([P, N_CAPS, PB], F32R, tag="esb")
                for h in range(2):
                    z_ps = z_ps_pool.tile([P, 4, PB], F32, tag="zps")
                    for mi in range(4):
                        m = 4 * h + mi
                        nc.tensor.matmul(
                            z_ps[:, mi, :],
                            kt_r[:, m * ND:(m + 1) * ND],
                            p_sb,
                            start=True,
                            stop=True,
                        )
                    p_bc = (
                        p_f.rearrange("p (o b) -> p o b", o=1)
                        .to_broadcast((P, 4, PB))
                    )
                    nc.vector.tensor_mul(
                        e_sb[:, 4 * h:4 * h + 4, :], z_ps[:], p_bc
                    )
                for ti in range(2):
                    for m in range(N_CAPS):
                        nc.tensor.matmul(
                            sg[:, ti * 64 + m * 8:ti * 64 + (m + 1) * 8],
                            e_sb[:, m, ti * P:(ti + 1) * P],
                            nmask_r[:],
                            start=True,
                            stop=True,
                        )
                nc.scalar.copy(
                    g_all[:, (t0 - tb) * 64:(t0 - tb + 2) * 64], sg[:, 0:128]
                )
                nc.scalar.copy(
                    sq_all[:, (t0 - tb) * 8:(t0 - tb + 2) * 8], sg[:, 128:144]
                )

            # ---------------- Routing (batched per group) ----------------
            def routing(tg0, tg1, lazy):
                Tg = tg1 - tg0
                TN = Tg * N_CAPS
                sq_g = sq_of[tg0][:]
                g_g = g_of[tg0][:]
                w_g = w_of[tg0][:]
                tga = f"_{tg0}"

                def bcast_tn(src, over):  # src [P, Tg*8] -> [P, Tg, 8, 8]
                    if over == "n":
                        ap = src.rearrange("p (t c o) -> p t c o", t=Tg, o=1)
                    else:
                        ap = src.rearrange("p (t o c) -> p t o c", t=Tg, o=1)
                    return ap.to_broadcast((P, Tg, 8, 8))

                def gamma_of(q, width, tagp):
                    tagp = tagp + tga
                    sq1 = rt.tile([P, width], F32, tag=f"ga{tagp}")
                    nc.scalar.activation(sq1[:], q, AF.Sqrt, bias=eps_sb[:, 0:1])
                    den = rt.tile([P, width], F32, tag=f"gc{tagp}")
                    nc.vector.scalar_tensor_tensor(
                        den[:], q, 1.0, sq1[:],
                        op0=mybir.AluOpType.add, op1=mybir.AluOpType.mult,
                    )
                    rden = rt.tile([P, width], F32, tag=f"gd{tagp}")
                    nc.vector.reciprocal(rden[:], den[:])
                    gam = rt.tile([P, width], F32, tag=f"ge{tagp}")
                    nc.vector.tensor_mul(gam[:], q, rden[:])
                    return gam

                gp = gamma_of(sq_g, TN, "p")

                g_v = g_g.rearrange("p (t m n) -> p t m n", t=Tg, m=8)
                sc_eng = nc.gpsimd if lazy else nc.vector
                sc_eng.tensor_mul(g_v, g_v, bcast_tn(gp[:], "n"))
                sc_eng.tensor_mul(g_v, g_v, bcast_tn(gp[:], "m"))

                blog = rt.tile([P, TN], F32, tag="blog" + tga)
                c_t = rt.tile([P, TN], F32, tag="ct" + tga)
                gam = None
                c_uniform = True
                for i in range(n_routing):
                    if i > 0:
                        e_t = rt.tile([P, TN], F32, tag="et" + tga)
                        nc.scalar.activation(e_t[:], blog[:], AF.Exp)
                        ssum = rt.tile([P, Tg], F32, tag="ssum" + tga)
                        nc.vector.reduce_sum(
                            ssum[:],
                            e_t[:].rearrange("p (t n) -> p t n", t=Tg),
                            axis=AX.X,
                        )
                        rsum = rt.tile([P, Tg], F32, tag="rsum" + tga)
                        nc.vector.reciprocal(rsum[:], ssum[:])
                        r_b = (
                            rsum[:].rearrange("p (t o) -> p t o", o=1)
                            .to_broadcast((P, Tg, 8))
                        )
                        nc.vector.tensor_mul(
                            c_t[:].rearrange("p (t n) -> p t n", t=Tg),
                            e_t[:].rearrange("p (t n) -> p t n", t=Tg),
                            r_b,
                        )
                        c_uniform = False

                    g_i = rt.tile([P, TN], F32, tag="gi" + tga)
                    if c_uniform:
                        nc.vector.reduce_sum(
                            g_i[:].rearrange("p (t n) -> p t n", t=Tg),
                            g_g.rearrange("p (t m n) -> p t n m", t=Tg, m=8),
                            axis=AX.X,
                        )
                        nc.vector.tensor_scalar_mul(g_i[:], g_i[:], 1.0 / N_CAPS)
                    else:
                        gc = rt.tile([P, Tg * 64], F32, tag="gcb" + tga)
                        eng = nc.gpsimd if lazy else nc.vector
                        eng.tensor_mul(
                            gc[:].rearrange("p (t m n) -> p t m n", t=Tg, m=8),
                            g_v,
                            bcast_tn(c_t[:], "n"),
                        )
                        nc.vector.reduce_sum(
                            g_i[:].rearrange("p (t n) -> p t n", t=Tg),
                            gc[:].rearrange("p (t m n) -> p t n m", t=Tg, m=8),
                            axis=AX.X,
                        )

                    q_t = rt.tile([P, Tg], F32, tag="qt" + tga)
                    if c_uniform:
                        nc.vector.reduce_sum(
                            q_t[:],
                            g_i[:].rearrange("p (t n) -> p t n", t=Tg),
                            axis=AX.X,
                        )
                        nc.vector.tensor_scalar_mul(q_t[:], q_t[:], 1.0 / N_CAPS)
                    else:
                        cg = rt.tile([P, TN], F32, tag="cg" + tga)
                        nc.vector.tensor_mul(cg[:], c_t[:], g_i[:])
                        nc.vector.reduce_sum(
                            q_t[:],
                            cg[:].rearrange("p (t n) -> p t n", t=Tg),
                            axis=AX.X,
                        )

                    if i < n_routing - 1:
                        gam = gamma_of(q_t[:], Tg, "q")
                        gam_b = (
                            gam[:].rearrange("p (t o) -> p t o", o=1)
                            .to_broadcast((P, Tg, 8))
                        )
                        if i == 0:
                            nc.vector.tensor_mul(
                                blog[:].rearrange("p (t n) -> p t n", t=Tg),
                                g_i[:].rearrange("p (t n) -> p t n", t=Tg),
                                gam_b,
                            )
                        else:
                            gg = rt.tile([P, TN], F32, tag="gg" + tga)
                            nc.vector.tensor_mul(
                                gg[:].rearrange("p (t n) -> p t n", t=Tg),
                                g_i[:].rearrange("p (t n) -> p t n", t=Tg),
                                gam_b,
                            )
                            nc.vector.tensor_add(blog[:], blog[:], gg[:])
                    else:
                        # w' = c * gp; the last gamma is applied as an ACT
                        # per-partition scale on the stage-2 output copy, so
                        # stage 2 starts ~6 chain steps earlier.
                        if c_uniform:
                            nc.vector.tensor_scalar_mul(
                                w_g, gp[:], 1.0 / N_CAPS
                            )
                        else:
                            nc.vector.tensor_mul(w_g, c_t[:], gp[:])
                        gam = gamma_of(q_t[:], Tg, "q")
                        nc.vector.tensor_copy(gam_of[tg0][:], gam[:])

            # ---------------- Stage 2 (per pair of tiles) ----------------
            v_blocks = {}

            def stage2(pr, pools):
                # PSUM comes from the stage-1 pools (shared tags) so there is
                # no pool-handover barrier between phases.
                xt_ps_pool, p_ps_pool, z_ps_pool, sg_ps_pool = pools
                t0 = 2 * pr
                tb = grp_base(pr)
                w_all = w_of[tb]
                gam_all = gam_of[tb]
                wt_ps = xt_ps_pool.tile([P, PB], F32, tag="xtps")
                for ti in range(2):
                    nc.tensor.transpose(
                        wt_ps[0:N_CAPS, ti * P:(ti + 1) * P],
                        w_all[:, (t0 - tb + ti) * N_CAPS:(t0 - tb + ti + 1) * N_CAPS],
                        ident[:],
                    )
                wt_sb = s2sb.tile([N_CAPS, PB], F32R, tag="wtsb")
                nc.scalar.copy(wt_sb[:], wt_ps[0:N_CAPS, :])

                wb_ps = xt_ps_pool.tile([P, PB], F32, tag="xtps")
                for ti in range(2):
                    nc.tensor.matmul(
                        wb_ps[:, ti * P:(ti + 1) * P],
                        mmask_r[:],
                        wt_sb[:, ti * P:(ti + 1) * P],
                        start=True,
                        stop=True,
                    )

                pw_sb = s2sb.tile([P, PB], F32R, tag="pwsb")
                nc.vector.tensor_mul(
                    pw_sb[:], p_all[:, t0 * ND:(t0 + 2) * ND].bitcast(F32), wb_ps[:]
                )

                v_ps4 = z_ps_pool.tile([P, 4, PB], F32, tag="zps")
                v_ps = v_ps4[:].rearrange("p a b -> p (a b)")
                for ti in range(2):
                    nc.tensor.matmul(
                        v_ps[:, ti * OUT_DIM:(ti + 1) * OUT_DIM],
                        pw_sb[:, ti * P:(ti + 1) * P],
                        wflat_r[:],
                        start=True,
                        stop=True,
                    )

                blk, half = pr // 2, pr % 2
                if half == 0:
                    v_blocks[blk] = s2sb.tile([P, 4, OUT_DIM], F32, tag="vsb", name=f"vsb{blk}")
                v_sb = v_blocks[blk]
                for ti in range(2):
                    # deferred squash scale gamma(t) applied per partition (=b)
                    nc.scalar.activation(
                        v_sb[:, 2 * half + ti, :],
                        v_ps[:, ti * OUT_DIM:(ti + 1) * OUT_DIM],
                        AF.Copy,
                        scale=gam_all[:, t0 - tb + ti:t0 - tb + ti + 1],
                    )
                lastpr = pr == n_pairs - 1
                if half == 1 or lastpr:
                    nq = 2 * half + 2
                    t00 = 4 * blk
                    nc.sync.dma_start(
                        v_ext[t00 * P:(t00 + nq) * P, :].rearrange(
                            "(q p) o -> p q o", p=P
                        ),
                        v_sb[:, 0:nq, :],
                    )

            # ---------------- Emission ----------------
            with (
                tc.tile_pool(name="xt_ps", bufs=2, space="PSUM") as xt_ps_pool,
                tc.tile_pool(name="p_ps", bufs=1, space="PSUM") as p_ps_pool,
                tc.tile_pool(name="z_ps", bufs=2, space="PSUM") as z_ps_pool,
                tc.tile_pool(name="sg_ps", bufs=1, space="PSUM") as sg_ps_pool,
            ):
                s1pools = (xt_ps_pool, p_ps_pool, z_ps_pool, sg_ps_pool)
                for gi, g in enumerate(groups):
                    for pr in g:
                        stage1(pr, s1pools)
                    # routing emitted right after its group's stage 1 so its
                    # (long, serial) chain overlaps later groups' stage 1
                    routing(2 * g[0], 2 * (g[-1] + 1),
                            lazy=(gi < len(groups) - 1))
                for g in groups:
                    for pr in g:
                        stage2(pr, s1pools)

    _split_multi_waits(nc)
    return nc



def _host_consts(Wp, bp, W):
    Wp = np.asarray(Wp, dtype=np.float32)
    bp = np.asarray(bp, dtype=np.float32)
    W = np.asarray(W, dtype=np.float32)
    wflat = W.reshape(ND, OUT_DIM)
    Kmat = wflat @ wflat.T                                    # [128, 128]
    # wpc[p, k*128+j] = Wp[k*128+p, j]
    wpc = Wp.reshape(K_CHUNKS, P, P).transpose(1, 0, 2).reshape(P, IN_DIM)
    # kt[(n'd'), m*128+(nd)] = delta_{n' m} * K[nd, m*16+d']  (block-masked lhsT)
    kt = np.zeros((ND, N_CAPS * ND), dtype=np.float32)
    for m in range(N_CAPS):
        kt[m * CAP_DIM:(m + 1) * CAP_DIM, m * ND:(m + 1) * ND] = \
            Kmat[:, m * CAP_DIM:(m + 1) * CAP_DIM].T
    nmask = np.zeros((ND, N_CAPS), dtype=np.float32)
    for n in range(N_CAPS):
        nmask[n * CAP_DIM:(n + 1) * CAP_DIM, n] = 1.0
    mmask = np.zeros((N_CAPS, ND), dtype=np.float32)
    for m in range(N_CAPS):
        mmask[m, m * CAP_DIM:(m + 1) * CAP_DIM] = 1.0
    cpack = np.zeros((P, 2825), dtype=np.float32)
    cpack[:, 0:128] = np.eye(P, dtype=np.float32)
    cpack[:, 128] = bp.reshape(ND)
    cpack[:, 129:137] = nmask
    cpack[0:8, 137:265] = mmask
    cpack[:, 265:1289] = wpc
    cpack[:, 1289:2313] = kt
    cpack[:, 2313:2825] = wflat
    return {"cpack": cpack}


_NC_CACHE = {}
TRACE = False
LAST_RESULT = None


def make_xt(x_part):
    """[rows, 1024] -> pre-transposed 2-pair-block layout:
    out[blk*128+p, k*512 + pp*256 + b] = x_part[(2*blk+pp)*256+b, k*128+p]."""
    rows = x_part.shape[0]
    n_pairs = rows // PB
    n_blocks = (n_pairs + 1) // 2
    xv = x_part.reshape(n_pairs, PB, K_CHUNKS, P)
    if n_pairs % 2:
        xv = np.concatenate([xv, np.zeros_like(xv[:1])], axis=0)
    t = xv.reshape(n_blocks, 2, PB, K_CHUNKS, P).transpose(0, 4, 3, 1, 2)
    return np.ascontiguousarray(t.reshape(n_blocks * P, K_CHUNKS * 2 * PB))


def kernel(x, Wp, bp, W, n_routing):
    n_routing = int(n_routing)
    x = np.ascontiguousarray(np.asarray(x, dtype=np.float32))
    assert x.shape == (B_FULL, IN_DIM)

    key = (n_routing,)
    if key not in _NC_CACHE:
        _NC_CACHE[key] = build_nc(n_routing)
    nc = _NC_CACHE[key]

    consts = _host_consts(Wp, bp, W)
    in_maps = []
    for c in range(N_CORES):
        m = {"xt": make_xt(x[c * B_CORE:(c + 1) * B_CORE, :])}
        m.update(consts)
        in_maps.append(m)

    global LAST_RESULT
    res = run_bass_kernel_spmd(nc, in_maps, list(range(N_CORES)), trace=TRACE)
    LAST_RESULT = res
    out = np.concatenate([res.results[c]["v"] for c in range(N_CORES)], axis=0)
    return out.astype(np.float32)



# revision 17
# speedup vs baseline: 1.1453x; 1.1453x over previous
"""CapsuleNetwork Trainium2 kernel (Bass/Tile), 8-core data parallel.

Math reformulation (validated vs reference in fp32, rel err ~3e-7):
  primary p = x @ Wp + bp, viewed [B, n=8, d=16]
  squash scales gp[b,n] = gamma(||p_n||^2),  gamma(q) = q/(1+q)/sqrt(q+1e-8)
  u_hat_n = gp_n * (p_n @ W_n)
  Routing only needs the per-sample Gram matrix
      G[b,n,m] = u_hat_n . u_hat_m = gp_n gp_m * (p_n K_nm p_m),
  with K = Wflat @ Wflat^T precomputed on host ([128,128]).
  Routing runs on [B,8]/[B,8,8] tensors; final
      v = sum_m w_m u_hat_m = (p .* w_bcast) @ Wflat,  w = gam_last*c_last*gp.

Per-core: 2048 rows = 16 tiles of 128, processed as 8 pairs (256-wide
matmuls).  Tiles are split into 2 groups; each group's routing (DVE-only,
on [128, wide] batched tensors) and stage-2 output overlap the other
group's PE-heavy stage 1.
"""

import numpy as np

import concourse.bass as bass
import concourse.tile as tile
from concourse import mybir
from concourse.bass_utils import run_bass_kernel_spmd
from concourse.vector_clock import ScopedClock

F32 = mybir.dt.float32
BF16 = mybir.dt.bfloat16   # 1 cyc/row on PE at any free width; halves DMA bytes
AF = mybir.ActivationFunctionType
AX = mybir.AxisListType

N_CORES = 8
B_FULL, IN_DIM, OUT_DIM = 16384, 1024, 512
N_CAPS, CAP_DIM = 8, 16
ND = N_CAPS * CAP_DIM          # 128
B_CORE = B_FULL // N_CORES     # 2048
P = 128                        # partitions / tile rows
PB = 2 * P                     # batch per pair
K_CHUNKS = IN_DIM // P         # 8


def _patched_drain_and_barrier(self, tick_clock, wait_clock):
    # Walrus in this env allows at most ONE sem wait per instruction; the
    # stock tail drain accumulates one wait per live sem.  Collect waits on
    # a NOP, then re-emit one wait per NOP.
    nc = self.nc
    collector = nc.sync.nop()
    wait_clock.add_sem_waits(
        collector.ins, ScopedClock({None: tick_clock.global_clock})
    )
    si = collector.ins.sync_info
    waits = list(si.on_wait) if si is not None else []
    if len(waits) > 1:
        collector.ins.sync_info = mybir.SyncInfo(on_wait=waits[:1], on_update=[])
        for w in waits[1:]:
            n = nc.sync.nop()
            n.ins.sync_info = mybir.SyncInfo(on_wait=[w], on_update=[])
    nc.sync.drain()
    nc.all_engine_barrier()
    popped = nc._tile_sem_poison_stack.pop()
    assert popped is self._sem_poison
    nc.clear_and_free_semaphores(list(self.sems.allocated().values()))
    nc.all_engine_barrier()


tile.TileContext._drain_and_barrier = _patched_drain_and_barrier


def _split_multi_waits(nc):
    """Walrus here accepts at most one sem wait per instruction.  Tile's
    wait-assignment can attach several; split the extras onto single-wait
    NOPs inserted just before the instruction on the same engine."""
    k = 0
    for fn in nc.m.functions:
        for blk in fn.blocks:
            out = []
            for inst in blk.instructions:
                si = inst.sync_info
                if si is not None and len(si.on_wait) > 1:
                    waits = list(si.on_wait)
                    for w in waits[:-1]:
                        nop = mybir.InstNoOp(name=f"wsplit-{k}", ins=[], outs=[])
                        k += 1
                        nop.engine = inst.engine
                        nop.sync_info = mybir.SyncInfo(on_wait=[w], on_update=[])
                        nc.register_instruction(nop, overwrite=True)
                        out.append(nop)
                    inst.sync_info = mybir.SyncInfo(
                        on_wait=[waits[-1]], on_update=list(si.on_update)
                    )
                out.append(inst)
            blk.instructions = out

def build_nc(n_routing: int, n_tiles: int = B_CORE // P):
    assert n_tiles % 2 == 0
    nc = bass.Bass()
    rows = n_tiles * P
    n_pairs = n_tiles // 2
    if n_pairs >= 2:
        h = (n_pairs + 1) // 2
        groups = [list(range(0, h)), list(range(h, n_pairs))]
    else:
        groups = [list(range(n_pairs))]
    groups = [g for g in groups if g]

    # xt: host-pre-transposed x, block layout [blk, p, k, pp, b] so a
    # 2-pair block is one contiguous [128, 8*512] DMA
    n_blocks = (n_pairs + 1) // 2
    xt_ext = nc.declare_dram_parameter(
        "xt", [n_blocks * P, K_CHUNKS * 2 * PB], BF16, isOutput=False
    )
    # packed consts, one tensor (bf16): cols [0:128]=ident, [128]=bp,
    # [129:137]=nmask, [137:265]=mmask(rows 0:8), [265:1289]=wpc,
    # [1289:2313]=kt, [2313:2825]=wflat
    CPACK = 2825
    cp_ext = nc.declare_dram_parameter("cpack", [P, CPACK], BF16, isOutput=False)
    v_ext = nc.declare_dram_parameter("v", [rows, OUT_DIM], BF16, isOutput=True)

    T = n_tiles

    with tile.TileContext(nc) as tc:
        with (
            tc.tile_pool(name="consts", bufs=1) as cpool,
            tc.tile_pool(name="persist", bufs=1) as ppool,
            tc.tile_pool(name="xin", bufs=2) as xpool,
            tc.tile_pool(name="s1sb", bufs=2) as s1pool,
            tc.tile_pool(name="rt", bufs=1) as rt,
            tc.tile_pool(name="s2sb", bufs=2) as s2sb,
        ):
            cp = cpool.tile([P, CPACK], BF16)
            nc.sync.dma_start(cp[:, 0:1289], cp_ext[:, 0:1289])
            ident = cp[:, 0:128]
            bp_sb = cp[:, 128:129]
            nmask_r = cp[:, 129:137]
            mmask_r = cp[0:8, 137:265]
            wpc_r = cp[:, 265:1289]
            eps_sb = cpool.tile([P, 1], F32)
            nc.gpsimd.memset(eps_sb[:], 1e-8)

            # xt prefetch for the first two 2-pair blocks, between const DMAs
            x_pre = {}

            def load_xt_block(blk):
                xt_sb = s1pool.tile([P, K_CHUNKS, 2 * PB], BF16, tag="xt")
                xin = xt_ext[blk * P:(blk + 1) * P, :].rearrange(
                    "p (k c) -> p k c", k=K_CHUNKS
                )
                if blk == 0:
                    nc.scalar.dma_start(xt_sb[:, 0:2, :], xin[:, 0:2, :])
                    nc.scalar.dma_start(xt_sb[:, 2:5, :], xin[:, 2:5, :])
                    nc.scalar.dma_start(
                        xt_sb[:, 5:K_CHUNKS, :], xin[:, 5:K_CHUNKS, :]
                    )
                elif blk == 1:
                    nc.scalar.dma_start(xt_sb[:, 0:4, :], xin[:, 0:4, :])
                    nc.scalar.dma_start(
                        xt_sb[:, 4:K_CHUNKS, :], xin[:, 4:K_CHUNKS, :]
                    )
                else:
                    nc.scalar.dma_start(xt_sb[:], xin)
                return xt_sb

            for blk in range(min(2, n_blocks)):
                x_pre[blk] = load_xt_block(blk)

            nc.sync.dma_start(cp[:, 1289:CPACK], cp_ext[:, 1289:CPACK])
            kt_r = cp[:, 1289:2313]
            wflat_r = cp[:, 2313:2825]

            # Wide per-core accumulators; sq/g/w/gam are PER GROUP so group
            # chains share no tensors at all
            p_all = ppool.tile([P, T * ND], BF16)       # primary^T, [:, t*128:...] = [(nd), b]
            t_of_g = {}
            sq_of, g_of, w_of, gam_of = {}, {}, {}, {}
            for gi, g in enumerate(groups):
                tg0, tg1 = 2 * g[0], 2 * (g[-1] + 1)
                t_of_g[gi] = (tg0, tg1)
                Tg = tg1 - tg0
                sq_of[tg0] = ppool.tile([P, Tg * N_CAPS], F32, name=f"sqg{tg0}")
                g_of[tg0] = ppool.tile([P, Tg * 64], F32, name=f"gg{tg0}")
                w_of[tg0] = ppool.tile([P, Tg * N_CAPS], BF16, name=f"wg{tg0}")
                gam_of[tg0] = ppool.tile([P, Tg], F32, name=f"gamg{tg0}")

            # ---------------- Stage 1 (per pair of tiles) ----------------
            def grp_base(pr):
                for gi, g in enumerate(groups):
                    if pr in g:
                        return t_of_g[gi][0]
                raise AssertionError

            xt_blocks = {}

            def stage1(pr, pools):
                xt_ps_pool, p_ps_pool, z_ps_pool, sg_ps_pool = pools
                t0 = 2 * pr
                tb = grp_base(pr)
                g_all = g_of[tb]
                sq_all = sq_of[tb]
                blk, half = pr // 2, pr % 2
                if half == 0:
                    if blk in x_pre:
                        xt_blocks[blk] = x_pre[blk]
                    else:
                        xt_blocks[blk] = load_xt_block(blk)
                xt_sb = xt_blocks[blk][:, :, half * PB:(half + 1) * PB]

                p_ps = p_ps_pool.tile([P, PB], F32, tag="pps")
                for k in range(K_CHUNKS):
                    nc.tensor.matmul(
                        p_ps[:],
                        wpc_r[:, k * P:(k + 1) * P],
                        xt_sb[:, k, :],
                        start=(k == 0),
                        stop=(k == K_CHUNKS - 1),
                    )
                p_sb = p_all[:, t0 * ND:(t0 + 2) * ND]   # [(nd), 256] bf16
                nc.scalar.activation(
                    p_sb, p_ps[:], AF.Identity, bias=bp_sb[:, 0:1], scale=1.0
                )

                # all-bf16 SBUF operands -> DVE 2x mode (~193ns)
                p2_sb = s1pool.tile([P, PB], BF16, tag="p2")
                nc.vector.tensor_mul(p2_sb[:], p_sb, p_sb)

                # sq & G share one PSUM tile: [:, 0:128] = G(2 tiles), [:, 128:144] = sq
                sg = sg_ps_pool.tile([P, 144], F32, tag="sg")
                for ti in range(2):
                    nc.tensor.matmul(
                        sg[:, 128 + ti * 8:128 + (ti + 1) * 8],
                        p2_sb[:, ti * P:(ti + 1) * P],
                        nmask_r[:],
                        start=True,
                        stop=True,
                    )

                e_sb = s1pool.tile([P, N_CAPS, PB], BF16, tag="esb")
                for h in range(2):
                    z_ps = z_ps_pool.tile([P, 4, PB], F32, tag="zps")
                    for mi in range(4):
                        m = 4 * h + mi
                        nc.tensor.matmul(
                            z_ps[:, mi, :],
                            kt_r[:, m * ND:(m + 1) * ND],
                            p_sb,
                            start=True,
                            stop=True,
                        )
                    p_bc = (
                        p_sb.rearrange("p (o b) -> p o b", o=1)
                        .to_broadcast((P, 4, PB))
                    )
                    nc.vector.tensor_mul(
                        e_sb[:, 4 * h:4 * h + 4, :], z_ps[:], p_bc
                    )
                for ti in range(2):
                    for m in range(N_CAPS):
                        nc.tensor.matmul(
                            sg[:, ti * 64 + m * 8:ti * 64 + (m + 1) * 8],
                            e_sb[:, m, ti * P:(ti + 1) * P],
                            nmask_r[:],
                            start=True,
                            stop=True,
                        )
                nc.scalar.copy(
                    g_all[:, (t0 - tb) * 64:(t0 - tb + 2) * 64], sg[:, 0:128]
                )
                nc.scalar.copy(
                    sq_all[:, (t0 - tb) * 8:(t0 - tb + 2) * 8], sg[:, 128:144]
                )

            # ---------------- Routing (batched per group) ----------------
            def routing(tg0, tg1, lazy):
                Tg = tg1 - tg0
                TN = Tg * N_CAPS
                sq_g = sq_of[tg0][:]
                g_g = g_of[tg0][:]
                w_g = w_of[tg0][:]
                tga = f"_{tg0}"

                def bcast_tn(src, over):  # src [P, Tg*8] -> [P, Tg, 8, 8]
                    if over == "n":
                        ap = src.rearrange("p (t c o) -> p t c o", t=Tg, o=1)
                    else:
                        ap = src.rearrange("p (t o c) -> p t o c", t=Tg, o=1)
                    return ap.to_broadcast((P, Tg, 8, 8))

                def gamma_of(q, width, tagp):
                    tagp = tagp + tga
                    sq1 = rt.tile([P, width], F32, tag=f"ga{tagp}")
                    nc.scalar.activation(sq1[:], q, AF.Sqrt, bias=eps_sb[:, 0:1])
                    den = rt.tile([P, width], F32, tag=f"gc{tagp}")
                    nc.vector.scalar_tensor_tensor(
                        den[:], q, 1.0, sq1[:],
                        op0=mybir.AluOpType.add, op1=mybir.AluOpType.mult,
                    )
                    rden = rt.tile([P, width], F32, tag=f"gd{tagp}")
                    nc.vector.reciprocal(rden[:], den[:])
                    gam = rt.tile([P, width], F32, tag=f"ge{tagp}")
                    nc.vector.tensor_mul(gam[:], q, rden[:])
                    return gam

                gp = gamma_of(sq_g, TN, "p")

                g_v = g_g.rearrange("p (t m n) -> p t m n", t=Tg, m=8)
                sc_eng = nc.gpsimd if lazy else nc.vector
                sc_eng.tensor_mul(g_v, g_v, bcast_tn(gp[:], "n"))
                sc_eng.tensor_mul(g_v, g_v, bcast_tn(gp[:], "m"))

                blog = rt.tile([P, TN], F32, tag="blog" + tga)
                c_t = rt.tile([P, TN], F32, tag="ct" + tga)
                gam = None
                c_uniform = True
                for i in range(n_routing):
                    if i > 0:
                        e_t = rt.tile([P, TN], F32, tag="et" + tga)
                        nc.scalar.activation(e_t[:], blog[:], AF.Exp)
                        ssum = rt.tile([P, Tg], F32, tag="ssum" + tga)
                        nc.vector.reduce_sum(
                            ssum[:],
                            e_t[:].rearrange("p (t n) -> p t n", t=Tg),
                            axis=AX.X,
                        )
                        rsum = rt.tile([P, Tg], F32, tag="rsum" + tga)
                        nc.vector.reciprocal(rsum[:], ssum[:])
                        r_b = (
                            rsum[:].rearrange("p (t o) -> p t o", o=1)
                            .to_broadcast((P, Tg, 8))
                        )
                        nc.vector.tensor_mul(
                            c_t[:].rearrange("p (t n) -> p t n", t=Tg),
                            e_t[:].rearrange("p (t n) -> p t n", t=Tg),
                            r_b,
                        )
                        c_uniform = False

                    g_i = rt.tile([P, TN], F32, tag="gi" + tga)
                    if c_uniform:
                        nc.vector.reduce_sum(
                            g_i[:].rearrange("p (t n) -> p t n", t=Tg),
                            g_g.rearrange("p (t m n) -> p t n m", t=Tg, m=8),
                            axis=AX.X,
                        )
                        nc.vector.tensor_scalar_mul(g_i[:], g_i[:], 1.0 / N_CAPS)
                    else:
                        gc = rt.tile([P, Tg * 64], F32, tag="gcb" + tga)
                        eng = nc.gpsimd if lazy else nc.vector
                        eng.tensor_mul(
                            gc[:].rearrange("p (t m n) -> p t m n", t=Tg, m=8),
                            g_v,
                            bcast_tn(c_t[:], "n"),
                        )
                        nc.vector.reduce_sum(
                            g_i[:].rearrange("p (t n) -> p t n", t=Tg),
                            gc[:].rearrange("p (t m n) -> p t n m", t=Tg, m=8),
                            axis=AX.X,
                        )

                    q_t = rt.tile([P, Tg], F32, tag="qt" + tga)
                    if c_uniform:
                        nc.vector.reduce_sum(
                            q_t[:],
                            g_i[:].rearrange("p (t n) -> p t n", t=Tg),
                            axis=AX.X,
                        )
                        nc.vector.tensor_scalar_mul(q_t[:], q_t[:], 1.0 / N_CAPS)
                    else:
                        cg = rt.tile([P, TN], F32, tag="cg" + tga)
                        nc.vector.tensor_mul(cg[:], c_t[:], g_i[:])
                        nc.vector.reduce_sum(
                            q_t[:],
                            cg[:].rearrange("p (t n) -> p t n", t=Tg),
                            axis=AX.X,
                        )

                    if i < n_routing - 1:
                        gam = gamma_of(q_t[:], Tg, "q")
                        gam_b = (
                            gam[:].rearrange("p (t o) -> p t o", o=1)
                            .to_broadcast((P, Tg, 8))
                        )
                        if i == 0:
                            nc.vector.tensor_mul(
                                blog[:].rearrange("p (t n) -> p t n", t=Tg),
                                g_i[:].rearrange("p (t n) -> p t n", t=Tg),
                                gam_b,
                            )
                        else:
                            gg = rt.tile([P, TN], F32, tag="gg" + tga)
                            nc.vector.tensor_mul(
                                gg[:].rearrange("p (t n) -> p t n", t=Tg),
                                g_i[:].rearrange("p (t n) -> p t n", t=Tg),
                                gam_b,
                            )
                            nc.vector.tensor_add(blog[:], blog[:], gg[:])
                    else:
                        # w' = c * gp; the last gamma is applied as an ACT
                        # per-partition scale on the stage-2 output copy, so
                        # stage 2 starts ~6 chain steps earlier.
                        if c_uniform:
                            nc.vector.tensor_scalar_mul(
                                w_g, gp[:], 1.0 / N_CAPS
                            )
                        else:
                            nc.vector.tensor_mul(w_g, c_t[:], gp[:])
                        gam = gamma_of(q_t[:], Tg, "q")
                        nc.vector.tensor_copy(gam_of[tg0][:], gam[:])

            # ---------------- Stage 2 (per pair of tiles) ----------------
            v_blocks = {}

            def stage2(pr, pools):
                # PSUM comes from the stage-1 pools (shared tags) so there is
                # no pool-handover barrier between phases.
                xt_ps_pool, p_ps_pool, z_ps_pool, sg_ps_pool = pools
                t0 = 2 * pr
                tb = grp_base(pr)
                w_all = w_of[tb]
                gam_all = gam_of[tb]
                wt_ps = xt_ps_pool.tile([P, PB], BF16, tag="xtps")
                for ti in range(2):
                    nc.tensor.transpose(
                        wt_ps[0:N_CAPS, ti * P:(ti + 1) * P],
                        w_all[:, (t0 - tb + ti) * N_CAPS:(t0 - tb + ti + 1) * N_CAPS],
                        ident[:],
                    )
                wt_sb = s2sb.tile([N_CAPS, PB], BF16, tag="wtsb")
                nc.scalar.copy(wt_sb[:], wt_ps[0:N_CAPS, :])

                wb_ps = xt_ps_pool.tile([P, PB], F32, tag="xtps")
                for ti in range(2):
                    nc.tensor.matmul(
                        wb_ps[:, ti * P:(ti + 1) * P],
                        mmask_r[:],
                        wt_sb[:, ti * P:(ti + 1) * P],
                        start=True,
                        stop=True,
                    )

                pw_sb = s2sb.tile([P, PB], BF16, tag="pwsb")
                nc.vector.tensor_mul(
                    pw_sb[:], p_all[:, t0 * ND:(t0 + 2) * ND], wb_ps[:]
                )

                v_ps4 = z_ps_pool.tile([P, 4, PB], F32, tag="zps")
                v_ps = v_ps4[:].rearrange("p a b -> p (a b)")
                for ti in range(2):
                    nc.tensor.matmul(
                        v_ps[:, ti * OUT_DIM:(ti + 1) * OUT_DIM],
                        pw_sb[:, ti * P:(ti + 1) * P],
                        wflat_r[:],
                        start=True,
                        stop=True,
                    )

                blk, half = pr // 2, pr % 2
                if half == 0:
                    v_blocks[blk] = s2sb.tile([P, 4, OUT_DIM], BF16, tag="vsb", name=f"vsb{blk}")
                v_sb = v_blocks[blk]
                for ti in range(2):
                    # deferred squash scale gamma(t) applied per partition (=b)
                    nc.scalar.activation(
                        v_sb[:, 2 * half + ti, :],
                        v_ps[:, ti * OUT_DIM:(ti + 1) * OUT_DIM],
                        AF.Copy,
                        scale=gam_all[:, t0 - tb + ti:t0 - tb + ti + 1],
                    )
                lastpr = pr == n_pairs - 1
                if half == 1 or lastpr:
                    nq = 2 * half + 2
                    t00 = 4 * blk
                    nc.sync.dma_start(
                        v_ext[t00 * P:(t00 + nq) * P, :].rearrange(
                            "(q p) o -> p q o", p=P
                        ),
                        v_sb[:, 0:nq, :],
                    )

            # ---------------- Emission ----------------
            with (
                tc.tile_pool(name="xt_ps", bufs=2, space="PSUM") as xt_ps_pool,
                tc.tile_pool(name="p_ps", bufs=1, space="PSUM") as p_ps_pool,
                tc.tile_pool(name="z_ps", bufs=2, space="PSUM") as z_ps_pool,
                tc.tile_pool(name="sg_ps", bufs=1, space="PSUM") as sg_ps_pool,
            ):
                s1pools = (xt_ps_pool, p_ps_pool, z_ps_pool, sg_ps_pool)
                for gi, g in enumerate(groups):
                    for pr in g:
                        stage1(pr, s1pools)
                    # routing emitted right after its group's stage 1 so its
                    # (long, serial) chain overlaps later groups' stage 1
                    routing(2 * g[0], 2 * (g[-1] + 1),
                            lazy=(gi < len(groups) - 1))
                for g in groups:
                    for pr in g:
                        stage2(pr, s1pools)

    _split_multi_waits(nc)
    return nc



def _host_consts(Wp, bp, W):
    Wp = np.asarray(Wp, dtype=np.float32)
    bp = np.asarray(bp, dtype=np.float32)
    W = np.asarray(W, dtype=np.float32)
    wflat = W.reshape(ND, OUT_DIM)
    Kmat = wflat @ wflat.T                                    # [128, 128]
    # wpc[p, k*128+j] = Wp[k*128+p, j]
    wpc = Wp.reshape(K_CHUNKS, P, P).transpose(1, 0, 2).reshape(P, IN_DIM)
    # kt[(n'd'), m*128+(nd)] = delta_{n' m} * K[nd, m*16+d']  (block-masked lhsT)
    kt = np.zeros((ND, N_CAPS * ND), dtype=np.float32)
    for m in range(N_CAPS):
        kt[m * CAP_DIM:(m + 1) * CAP_DIM, m * ND:(m + 1) * ND] = \
            Kmat[:, m * CAP_DIM:(m + 1) * CAP_DIM].T
    nmask = np.zeros((ND, N_CAPS), dtype=np.float32)
    for n in range(N_CAPS):
        nmask[n * CAP_DIM:(n + 1) * CAP_DIM, n] = 1.0
    mmask = np.zeros((N_CAPS, ND), dtype=np.float32)
    for m in range(N_CAPS):
        mmask[m, m * CAP_DIM:(m + 1) * CAP_DIM] = 1.0
    cpack = np.zeros((P, 2825), dtype=np.float32)
    cpack[:, 0:128] = np.eye(P, dtype=np.float32)
    cpack[:, 128] = bp.reshape(ND)
    cpack[:, 129:137] = nmask
    cpack[0:8, 137:265] = mmask
    cpack[:, 265:1289] = wpc
    cpack[:, 1289:2313] = kt
    cpack[:, 2313:2825] = wflat
    import ml_dtypes
    return {"cpack": cpack.astype(ml_dtypes.bfloat16)}


_NC_CACHE = {}
TRACE = False
LAST_RESULT = None


def make_xt(x_part):
    """[rows, 1024] -> pre-transposed 2-pair-block layout:
    out[blk*128+p, k*512 + pp*256 + b] = x_part[(2*blk+pp)*256+b, k*128+p]."""
    rows = x_part.shape[0]
    n_pairs = rows // PB
    n_blocks = (n_pairs + 1) // 2
    import ml_dtypes
    xv = x_part.reshape(n_pairs, PB, K_CHUNKS, P)
    if n_pairs % 2:
        xv = np.concatenate([xv, np.zeros_like(xv[:1])], axis=0)
    t = xv.reshape(n_blocks, 2, PB, K_CHUNKS, P).transpose(0, 4, 3, 1, 2)
    return np.ascontiguousarray(
        t.reshape(n_blocks * P, K_CHUNKS * 2 * PB).astype(ml_dtypes.bfloat16)
    )


def kernel(x, Wp, bp, W, n_routing):
    n_routing = int(n_routing)
    x = np.ascontiguousarray(np.asarray(x, dtype=np.float32))
    assert x.shape == (B_FULL, IN_DIM)

    key = (n_routing,)
    if key not in _NC_CACHE:
        _NC_CACHE[key] = build_nc(n_routing)
    nc = _NC_CACHE[key]

    consts = _host_consts(Wp, bp, W)
    in_maps = []
    for c in range(N_CORES):
        m = {"xt": make_xt(x[c * B_CORE:(c + 1) * B_CORE, :])}
        m.update(consts)
        in_maps.append(m)

    global LAST_RESULT
    res = run_bass_kernel_spmd(nc, in_maps, list(range(N_CORES)), trace=TRACE)
    LAST_RESULT = res
    out = np.concatenate(
        [np.asarray(res.results[c]["v"]) for c in range(N_CORES)], axis=0
    )
    return out.astype(np.float32)

